# revision 1
# baseline (speedup 1.0000x reference)
"""Deformable conv v2 Trainium2 kernel (8 NeuronCores, data-parallel over batch).

Layout per core (1 sample):
  x [64, 128, 128] f32 in, out [64, 64, 128] f32.
  - offset/mask convs: PE matmuls over im2col APs of a zero-padded x (fp32).
  - sampling weights / integer indices: DVE/ACT elementwise, wo on partitions.
  - gather table xq [16512, 256] bf16 in DRAM: row r=(y,x) holds the 4 bilinear
    corner pixel-vectors [x(y,x), x(y,x+1), x(y+1,x), x(y+1,x+1)] so one 512B
    indirect-DMA descriptor fetches all 4 corners of one sampling location.
  - main loop over (ho, k): 128-location indirect gather + 4 fused
    scalar_tensor_tensor corner MACs -> sampledT [wo, c] tiles.
  - PE transposes pair-of-taps tiles, then accumulates the [576]-deep output
    contraction in PSUM.
"""
import os
import numpy as np

import concourse.bass as bass
import concourse.mybir as mybir
import concourse.tile as tile
from concourse.tile import TileContext
from concourse.vector_clock import ScopedClock

F32 = mybir.dt.float32
BF16 = mybir.dt.bfloat16
I32 = mybir.dt.int32
F32R = mybir.dt.float32r

B, C, H, W = 8, 64, 128, 128
K = 3
K2 = 9
SH, SW = 2, 1
PH, PW = 1, 1
Ho, Wo = 64, 128
CO = 64
N_CORES = 8
HWs = H * W           # 16384
XQ_LEAD = 129          # xq row r = corners of pixel (r - XQ_LEAD)
XQ_ROWS = HWs + 384    # 129 lead + 16384 + tail pad = 16768
HP, WP = H + 2, W + 2  # padded image 130x130

_MAXW = 1


def _patch_tile_drain():
    def _patched(self, tick_clock, wait_clock):
        nc = self.nc
        probe = nc.sync.nop()
        wait_clock.add_sem_waits(probe.ins, ScopedClock({None: tick_clock.global_clock}))
        nc.sync.drain()
        nc.all_engine_barrier()
        assert self.sems is not None
        popped = nc._tile_sem_poison_stack.pop()
        assert popped is self._sem_poison
        nc.clear_and_free_semaphores(list(self.sems.allocated().values()))
        nc.all_engine_barrier()
    TileContext._drain_and_barrier = _patched


def _split_sync_waits(nc, maxw=_MAXW):
    for f in nc.m.functions:
        for bb in f.blocks:
            out = []
            for ins in bb.instructions:
                si = ins.sync_info
                waits = list(si.on_wait) if si and si.on_wait else []
                if len(waits) > maxw:
                    for i in range(0, len(waits) - maxw, maxw):
                        nop = mybir.InstNoOp(name=f"I-wsplit-{nc.next_id()}", ins=[], outs=[])
                        nop.engine = ins.engine
                        nop.sync_info = mybir.SyncInfo(on_wait=waits[i:i + maxw], on_update=[])
                        out.append(nop)
                    rem = waits[len(waits) - (len(waits) % maxw or maxw):]
                    si.on_wait = rem
                out.append(ins)
            bb.instructions = out


def _build_nc(n_ho):
    from concourse.masks import make_identity
    AF = mybir.ActivationFunctionType
    OP = mybir.AluOpType

    nc = bass.Bass(use_seq_codegen=True)
    x_in = nc.declare_dram_parameter("x", [C, H * W], F32R, isOutput=False)
    w_om_in = nc.declare_dram_parameter("w_om", [C, K2, 48], F32R, isOutput=False)
    b_om_in = nc.declare_dram_parameter("b_om", [48, 1], F32, isOutput=False)
    w_cv_in = nc.declare_dram_parameter("w_cv", [128, 5, CO], BF16, isOutput=False)
    out_ext = nc.declare_dram_parameter("out", [CO, Ho * Wo], F32, isOutput=True)
    dbg = os.environ.get("DEFC_DEBUG", "0") == "1"
    if dbg:
        d_wt = nc.declare_dram_parameter("d_wt", [128, Ho, 48], F32, isOutput=True)
        d_w = nc.declare_dram_parameter("d_w", [4, 128, Ho * K2], F32, isOutput=True)
        d_idx = nc.declare_dram_parameter("d_idx", [128, Ho * K2], I32, isOutput=True)
        d_g = nc.declare_dram_parameter("d_g", [128, K2, 256], F32, isOutput=True)
        d_acc = nc.declare_dram_parameter("d_acc", [128, 5, 128], F32, isOutput=True)
    xq = nc.declare_dram_parameter("xq", [XQ_ROWS, 256], BF16, isOutput=False)

    from contextlib import ExitStack
    ctx = ExitStack()
    with TileContext(nc) as tc, ctx:
        sing = ctx.enter_context(tc.tile_pool(name="sing", bufs=1))
        conv_ps = ctx.enter_context(tc.tile_pool(name="conv_ps", bufs=1, space="PSUM"))
        tr_ps = ctx.enter_context(tc.tile_pool(name="tr_ps", bufs=4, space="PSUM"))
        wt_ps = ctx.enter_context(tc.tile_pool(name="wt_ps", bufs=1, space="PSUM"))
        out_ps = ctx.enter_context(tc.tile_pool(name="out_ps", bufs=2, space="PSUM"))
        gp = ctx.enter_context(tc.tile_pool(name="gp", bufs=16))
        accp = ctx.enter_context(tc.tile_pool(name="accp", bufs=12))
        rhp = ctx.enter_context(tc.tile_pool(name="rhp", bufs=4))
        scr = ctx.enter_context(tc.tile_pool(name="scr", bufs=4))
        stg = ctx.enter_context(tc.tile_pool(name="stg", bufs=2))

        ident = sing.tile([128, 128], F32)
        make_identity(nc, ident[:])

        # ---- load weights ----
        w_om = sing.tile([C, K2, 48], F32R)
        nc.sync.dma_start(out=w_om[:], in_=w_om_in[:])
        b_om = sing.tile([48, 1], F32)
        nc.sync.dma_start(out=b_om[:], in_=b_om_in[:])
        w_cv = sing.tile([128, 5, CO], BF16)
        nc.sync.dma_start(out=w_cv[:], in_=w_cv_in[:])

        # ---- padded image (fp32) for convs ----
        xpad = sing.tile([C, HP * WP], F32R)
        nc.vector.memset(xpad[:].bitcast(F32), 0.0)
        xpad_rows = xpad[:].rearrange("c (h w) -> c h w", h=HP)
        nc.sync.dma_start(
            out=xpad_rows[:, 1:H + 1, 1:W + 1],
            in_=x_in[:].rearrange("c (h w) -> c h w", h=H),
        )

        # ---- offset/mask convs + transpose to w_t [wo, ho, ch] ----
        w_t = sing.tile([128, Ho, 48], F32)
        for t in range(16):
            ps = conv_ps.tile([48, 512], F32, tag="cps")
            for ki in range(K2):
                ky, kx = ki // 3, ki % 3
                rhs = bass.AP(
                    tensor=xpad.tensor,
                    offset=xpad[:].offset + (4 * t * SH + ky) * WP + kx,
                    ap=[list(xpad[:].ap[0]), [SH * WP, 4], [SW, 128]],
                )
                nc.tensor.matmul(
                    out=ps[0:41, :],
                    lhsT=w_om[:, ki, 0:41],
                    rhs=rhs,
                    start=(ki == 0),
                    stop=(ki == K2 - 1),
                )
            cstage = stg.tile([48, 4, 128], F32, tag="cstage")
            nc.scalar.activation(cstage[0:18, :, :], ps[0:18, :].rearrange("p (a b) -> p a b", a=4),
                                 AF.Identity, bias=b_om[0:18, :], scale=1.0)
            nc.scalar.activation(cstage[32:41, :, :], ps[32:41, :].rearrange("p (a b) -> p a b", a=4),
                                 AF.Sigmoid, bias=b_om[32:41, :], scale=1.0)
            for a in range(4):
                tp = wt_ps.tile([128, 48], F32, tag="wtp")
                nc.tensor.transpose(out=tp[:, 0:41], in_=cstage[0:41, a, :],
                                    identity=ident[0:41, 0:41])
                nc.vector.tensor_copy(out=w_t[:, 4 * t + a, 0:41], in_=tp[:, 0:41])

        # ---- sampling weights + gather indices ----
        # iotas
        i_ho_i = sing.tile([128, Ho], I32)
        nc.gpsimd.iota(i_ho_i[:], pattern=[[SH, Ho]], base=0, channel_multiplier=0)
        i_ho = sing.tile([128, Ho], F32)
        nc.vector.tensor_copy(out=i_ho[:], in_=i_ho_i[:])
        i_wo_i = sing.tile([128, 1], I32)
        nc.gpsimd.iota(i_wo_i[:], pattern=[[0, 1]], base=0, channel_multiplier=1)
        i_wo = sing.tile([128, 1], F32)
        nc.vector.tensor_copy(out=i_wo[:], in_=i_wo_i[:])

        py = sing.tile([128, Ho, K2], F32)
        px = sing.tile([128, Ho, K2], F32)
        # off_y channels 2k, off_x channels 2k+1 in w_t's ch dim
        for g in range(3):
            # ky group: k in {3g, 3g+1, 3g+2} -> off_y ch {6g, 6g+2, 6g+4}
            offy = bass.AP(tensor=w_t.tensor, offset=w_t[:].offset + 6 * g,
                          ap=[list(w_t[:].ap[0]), [48, Ho], [2, 3]])
            dsty = bass.AP(tensor=py.tensor, offset=py[:].offset + 3 * g,
                          ap=[list(py[:].ap[0]), [K2, Ho], [1, 3]])
            # py = off_y + (2*ho - PH + ky)
            nc.vector.tensor_tensor(out=dsty, in0=offy,
                                    in1=i_ho[:].to_broadcast([128, Ho, 3]), op=OP.add)
            nc.vector.tensor_scalar_add(dsty, dsty, float(g - PH))
            # kx group: k in {g, g+3, g+6} -> off_x ch {2g+1, 2g+7, 2g+13}
            offx = bass.AP(tensor=w_t.tensor, offset=w_t[:].offset + 2 * g + 1,
                          ap=[list(w_t[:].ap[0]), [48, Ho], [6, 3]])
            dstx = bass.AP(tensor=px.tensor, offset=px[:].offset + g,
                          ap=[list(px[:].ap[0]), [K2, Ho], [3, 3]])
            nc.vector.tensor_scalar_add(dstx, offx, i_wo[:, 0:1])
            nc.vector.tensor_scalar_add(dstx, dstx, float(g - PW))

        NF = Ho * K2  # 576
        pyf = py[:].rearrange("p a b -> p (a b)")
        pxf = px[:].rearrange("p a b -> p (a b)")

        def floor_frac(pos, fl, fr):
            fl_i = scr.tile([128, NF], I32, tag="scri")
            tmp = scr.tile([128, NF], F32, tag="scr")
            nc.vector.tensor_scalar_add(tmp[:], pos, 15.5)  # round-nearest(x+15.5)=floor(x)+16
            nc.vector.tensor_copy(out=fl_i[:], in_=tmp[:])   # f32 -> i32 trunc
            nc.vector.tensor_copy(out=fl[:], in_=fl_i[:])
            nc.vector.tensor_scalar_add(fl[:], fl[:], -16.0)
            nc.vector.tensor_tensor(out=fr[:], in0=pos, in1=fl[:], op=OP.subtract)

        y0f = sing.tile([128, NF], F32)
        fy = sing.tile([128, NF], F32)
        x0f = sing.tile([128, NF], F32)
        fx = sing.tile([128, NF], F32)
        floor_frac(pyf, y0f, fy)
        floor_frac(pxf, x0f, fx)

        def valid_mult(dst, v, lo, hi):
            a = scr.tile([128, NF], F32, tag="scr")
            nc.vector.tensor_scalar(a[:], v, lo, None, op0=OP.is_ge)
            nc.vector.tensor_tensor(out=dst, in0=dst, in1=a[:], op=OP.mult)
            nc.vector.tensor_scalar(a[:], v, hi, None, op0=OP.is_le)
            nc.vector.tensor_tensor(out=dst, in0=dst, in1=a[:], op=OP.mult)

        m_sig = sing.tile([128, NF], F32)
        msrc = bass.AP(tensor=w_t.tensor, offset=w_t[:].offset + 32,
                       ap=[list(w_t[:].ap[0]), [48, Ho], [1, K2]])
        nc.vector.tensor_copy(out=m_sig[:], in_=msrc)

        wy0 = sing.tile([128, NF], F32)
        nc.vector.tensor_scalar(wy0[:], fy[:], 1.0, -1.0, op0=OP.subtract, op1=OP.mult)
        nc.vector.tensor_tensor(out=wy0[:], in0=wy0[:], in1=m_sig[:], op=OP.mult)
        valid_mult(wy0[:], y0f[:], 0.0, float(H - 1))
        wy1 = sing.tile([128, NF], F32)
        nc.vector.tensor_tensor(out=wy1[:], in0=fy[:], in1=m_sig[:], op=OP.mult)
        valid_mult(wy1[:], y0f[:], -1.0, float(H - 2))
        wx0 = sing.tile([128, NF], F32)
        nc.vector.tensor_scalar(wx0[:], fx[:], 1.0, -1.0, op0=OP.subtract, op1=OP.mult)
        valid_mult(wx0[:], x0f[:], 0.0, float(W - 1))
        wx1 = sing.tile([128, NF], F32)
        nc.vector.tensor_copy(out=wx1[:], in_=fx[:])
        valid_mult(wx1[:], x0f[:], -1.0, float(W - 2))

        w00 = sing.tile([128, NF], F32)
        nc.vector.tensor_tensor(out=w00[:], in0=wy0[:], in1=wx0[:], op=OP.mult)
        w01 = sing.tile([128, NF], F32)
        nc.vector.tensor_tensor(out=w01[:], in0=wy0[:], in1=wx1[:], op=OP.mult)
        w10 = sing.tile([128, NF], F32)
        nc.vector.tensor_tensor(out=w10[:], in0=wy1[:], in1=wx0[:], op=OP.mult)
        w11 = sing.tile([128, NF], F32)
        nc.vector.tensor_tensor(out=w11[:], in0=wy1[:], in1=wx1[:], op=OP.mult)

        idx = sing.tile([128, NF], I32)
        idf = scr.tile([128, NF], F32, tag="scr")
        nc.vector.tensor_scalar(idf[:], y0f[:], float(W), float(XQ_LEAD), op0=OP.mult, op1=OP.add)
        nc.vector.tensor_tensor(out=idf[:], in0=idf[:], in1=x0f[:], op=OP.add)
        nc.vector.tensor_scalar(idf[:], idf[:], 0.0, float(XQ_ROWS - 1), op0=OP.max, op1=OP.min)
        nc.vector.tensor_copy(out=idx[:], in_=idf[:])

        if dbg:
            nc.sync.dma_start(out=d_wt[:], in_=w_t[:])
            for i, wt in enumerate((w00, w01, w10, w11)):
                nc.sync.dma_start(out=d_w[i], in_=wt[:])
            nc.sync.dma_start(out=d_idx[:], in_=idx[:])

        # ---- main loop ----
        for ho in range(n_ho):
            accs = []
            for half in range(5):
                a = accp.tile([128, 128], F32, tag="acc")
                accs.append(a)
            for ki in range(K2):
                s = ho * K2 + ki
                sl = slice(s, s + 1)
                g = gp.tile([128, 256], BF16, tag="g")
                nc.gpsimd.indirect_dma_start(
                    out=g[:], out_offset=None, in_=xq[:],
                    in_offset=bass.IndirectOffsetOnAxis(ap=idx[:, sl], axis=0),
                )
                acc = accs[ki // 2]
                col = 64 * (ki % 2)
                t = acc[:, col:col + 64]
                if ki % 2 == 0:
                    # ACT-assisted path
                    tm = rhp.tile([128, 256], F32, tag="tm")
                    nc.scalar.activation(tm[:, 0:64], g[:, 0:64], AF.Identity,
                                         scale=w00[:, sl])
                    nc.scalar.activation(tm[:, 64:128], g[:, 64:128], AF.Identity,
                                         scale=w01[:, sl])
                    nc.scalar.activation(tm[:, 128:192], g[:, 128:192], AF.Identity,
                                         scale=w10[:, sl])
                    nc.scalar.activation(tm[:, 192:256], g[:, 192:256], AF.Identity,
                                         scale=w11[:, sl])
                    u = rhp.tile([128, 128], F32, tag="u")
                    nc.vector.tensor_tensor(out=u[:], in0=tm[:, 0:128],
                                            in1=tm[:, 128:256], op=OP.add)
                    nc.vector.tensor_tensor(out=t, in0=u[:, 0:64], in1=u[:, 64:128],
                                            op=OP.add)
                else:
                    # DVE fused-MAC path
                    nc.vector.scalar_tensor_tensor(t, g[:, 0:64], w00[:, sl], g[:, 0:64],
                                                   op0=OP.mult, op1=OP.bypass)
                    nc.vector.scalar_tensor_tensor(t, g[:, 64:128], w01[:, sl], t,
                                                   op0=OP.mult, op1=OP.add)
                    nc.vector.scalar_tensor_tensor(t, g[:, 128:192], w10[:, sl], t,
                                                   op0=OP.mult, op1=OP.add)
                    nc.vector.scalar_tensor_tensor(t, g[:, 192:256], w11[:, sl], t,
                                                   op0=OP.mult, op1=OP.add)
                if dbg and ho == 0:
                    gf = rhp.tile([128, 256], F32, tag="gf")
                    nc.vector.tensor_copy(out=gf[:], in_=g[:])
                    nc.sync.dma_start(out=d_g[:, ki, :], in_=gf[:])
            if dbg and ho == 0:
                for half in range(5):
                    nc.sync.dma_start(out=d_acc[:, half, :], in_=accs[half][:])
            ops = out_ps.tile([CO, 128], F32, tag="ops")
            for half in range(5):
                tp = tr_ps.tile([128, 128], F32, tag="str")
                nc.tensor.transpose(out=tp[:], in_=accs[half][:], identity=ident[:])
                rh = rhp.tile([128, 128], BF16, tag="rh")
                if half % 2 == 0:
                    nc.vector.tensor_copy(out=rh[:], in_=tp[:])
                else:
                    nc.scalar.copy(out=rh[:], in_=tp[:])
                pdim = 128 if half < 4 else 64
                nc.tensor.matmul(
                    out=ops[:],
                    lhsT=w_cv[0:pdim, half, :],
                    rhs=rh[0:pdim, :],
                    start=(half == 0),
                    stop=(half == 4),
                )
            ob = rhp.tile([CO, 128], F32, tag="ob")
            nc.vector.tensor_copy(out=ob[:], in_=ops[:])
            nc.sync.dma_start(
                out=out_ext[:].rearrange("o (ho wo) -> o ho wo", ho=Ho)[:, ho, :],
                in_=ob[:],
            )

    _split_sync_waits(nc)
    return nc


_CACHED = {}


def _get_runner(n_ho):
    if n_ho in _CACHED:
        return _CACHED[n_ho]
    _patch_tile_drain()
    nc = _build_nc(n_ho)
    from concourse.bass_utils import run_bass_kernel_spmd
    def run(in_maps, trace=False):
        return run_bass_kernel_spmd(nc, in_maps, list(range(N_CORES)), trace=trace)
    _CACHED[n_ho] = run
    return run


def _prep_weights(w_offset, b_offset, w_mask, b_mask, w_conv):
    import ml_dtypes
    w_om = np.zeros((C, K2, 48), np.float32)
    for ki in range(K2):
        ky, kx = ki // 3, ki % 3
        w_om[:, ki, 0:18] = w_offset[:, :, ky, kx].T
        w_om[:, ki, 32:41] = w_mask[:, :, ky, kx].T
    b_om = np.zeros((48, 1), np.float32)
    b_om[0:18, 0] = b_offset
    b_om[32:41, 0] = b_mask
    w_cv = np.zeros((128, 5, CO), np.float32)
    for half in range(5):
        k0 = 2 * half
        w_cv[0:64, half, :] = w_conv[:, :, k0 // 3, k0 % 3].T
        if k0 + 1 < K2:
            k1 = k0 + 1
            w_cv[64:128, half, :] = w_conv[:, :, k1 // 3, k1 % 3].T
    return w_om, b_om, w_cv.astype(ml_dtypes.bfloat16)


def _build_xq(xs):
    import ml_dtypes
    xt = np.zeros((XQ_LEAD + HWs + 384, C), np.float32)
    xt[XQ_LEAD:XQ_LEAD + HWs] = xs.reshape(C, HWs).T
    r0 = XQ_LEAD
    xqf = np.stack([xt[r0 - 129:r0 - 129 + XQ_ROWS],
                    xt[r0 - 128:r0 - 128 + XQ_ROWS],
                    xt[r0 - 1:r0 - 1 + XQ_ROWS],
                    xt[r0:r0 + XQ_ROWS]], axis=1)
    return np.ascontiguousarray(xqf.reshape(XQ_ROWS, 256).astype(ml_dtypes.bfloat16))


def kernel(x, w_offset, b_offset, w_mask, b_mask, w_conv, _trace=False):
    x = np.ascontiguousarray(np.asarray(x, np.float32))
    w_om, b_om, w_cv = _prep_weights(
        np.asarray(w_offset, np.float32), np.asarray(b_offset, np.float32),
        np.asarray(w_mask, np.float32), np.asarray(b_mask, np.float32),
        np.asarray(w_conv, np.float32))
    n_ho = int(os.environ.get("DEFC_N_HO", Ho))
    run = _get_runner(n_ho)
    in_maps = [
        {"x": x[c].reshape(C, H * W), "w_om": w_om, "b_om": b_om, "w_cv": w_cv,
         "xq": _build_xq(x[c])}
        for c in range(N_CORES)
    ]
    res = run(in_maps, trace=_trace)
    out = np.stack([res.results[c]["out"].reshape(CO, Ho, Wo) for c in range(N_CORES)])
    kernel._last_exec_ns = getattr(res, "exec_time_ns", None)
    return out



# revision 11
# speedup vs baseline: 1.0649x; 1.0649x over previous
"""Deformable conv v2 Trainium2 kernel (8 NeuronCores, data-parallel over batch).

Layout per core (1 sample):
  x [64, 128, 128] f32 in, out [64, 64, 128] f32.
  - offset/mask convs: PE matmuls over im2col APs of a zero-padded x (fp32).
  - sampling weights / integer indices: DVE/ACT elementwise, wo on partitions.
  - gather table xq [16512, 256] bf16 in DRAM: row r=(y,x) holds the 4 bilinear
    corner pixel-vectors [x(y,x), x(y,x+1), x(y+1,x), x(y+1,x+1)] so one 512B
    indirect-DMA descriptor fetches all 4 corners of one sampling location.
  - main loop over (ho, k): 128-location indirect gather + 4 fused
    scalar_tensor_tensor corner MACs -> sampledT [wo, c] tiles.
  - PE transposes pair-of-taps tiles, then accumulates the [576]-deep output
    contraction in PSUM.
"""
import os
import numpy as np

import concourse.bass as bass
import concourse.mybir as mybir
import concourse.tile as tile
from concourse.tile import TileContext
from concourse.vector_clock import ScopedClock

F32 = mybir.dt.float32
BF16 = mybir.dt.bfloat16
I32 = mybir.dt.int32
F32R = mybir.dt.float32r

B, C, H, W = 8, 64, 128, 128
K = 3
K2 = 9
SH, SW = 2, 1
PH, PW = 1, 1
Ho, Wo = 64, 128
CO = 64
N_CORES = 8
HWs = H * W           # 16384
XQ_LEAD = 129          # xq row r = corners of pixel (r - XQ_LEAD)
XQ_ROWS = HWs + 384    # 129 lead + 16384 + tail pad = 16768
HP, WP = H + 2, W + 2  # padded image 130x130

_MAXW = 1


def _patch_tile_drain():
    def _patched(self, tick_clock, wait_clock):
        nc = self.nc
        probe = nc.sync.nop()
        wait_clock.add_sem_waits(probe.ins, ScopedClock({None: tick_clock.global_clock}))
        nc.sync.drain()
        nc.all_engine_barrier()
        assert self.sems is not None
        popped = nc._tile_sem_poison_stack.pop()
        assert popped is self._sem_poison
        nc.clear_and_free_semaphores(list(self.sems.allocated().values()))
        nc.all_engine_barrier()
    TileContext._drain_and_barrier = _patched


def _split_sync_waits(nc, maxw=_MAXW):
    for f in nc.m.functions:
        for bb in f.blocks:
            out = []
            for ins in bb.instructions:
                si = ins.sync_info
                waits = list(si.on_wait) if si and si.on_wait else []
                if len(waits) > maxw:
                    for i in range(0, len(waits) - maxw, maxw):
                        nop = mybir.InstNoOp(name=f"I-wsplit-{nc.next_id()}", ins=[], outs=[])
                        nop.engine = ins.engine
                        nop.sync_info = mybir.SyncInfo(on_wait=waits[i:i + maxw], on_update=[])
                        out.append(nop)
                    rem = waits[len(waits) - (len(waits) % maxw or maxw):]
                    si.on_wait = rem
                out.append(ins)
            bb.instructions = out


def _build_nc(n_ho):
    from concourse.masks import make_identity
    AF = mybir.ActivationFunctionType
    OP = mybir.AluOpType

    nc = bass.Bass(use_seq_codegen=True)
    x_in = nc.declare_dram_parameter("x", [C, H * W], F32R, isOutput=False)
    w_om_in = nc.declare_dram_parameter("w_om", [C, K2, 48], F32R, isOutput=False)
    b_om_in = nc.declare_dram_parameter("b_om", [48, 1], F32, isOutput=False)
    w_cv_in = nc.declare_dram_parameter("w_cv", [128, 5, CO], BF16, isOutput=False)
    out_ext = nc.declare_dram_parameter("out", [CO, Ho * Wo], F32, isOutput=True)
    dbg = os.environ.get("DEFC_DEBUG", "0") == "1"
    if dbg:
        d_wt = nc.declare_dram_parameter("d_wt", [128, Ho, 48], F32, isOutput=True)
        d_w = nc.declare_dram_parameter("d_w", [4, 128, Ho * K2], F32, isOutput=True)
        d_idx = nc.declare_dram_parameter("d_idx", [128, Ho * K2], I32, isOutput=True)
        d_g = nc.declare_dram_parameter("d_g", [128, K2, 256], F32, isOutput=True)
        d_acc = nc.declare_dram_parameter("d_acc", [128, 5, 128], F32, isOutput=True)
    xq = nc.declare_dram_parameter("xq", [XQ_ROWS, 256], BF16, isOutput=False)

    from contextlib import ExitStack
    ctx = ExitStack()
    with TileContext(nc) as tc, ctx:
        sing = ctx.enter_context(tc.tile_pool(name="sing", bufs=1))
        conv_ps = ctx.enter_context(tc.tile_pool(name="conv_ps", bufs=1, space="PSUM"))
        tr_ps = ctx.enter_context(tc.tile_pool(name="tr_ps", bufs=4, space="PSUM"))
        wt_ps = ctx.enter_context(tc.tile_pool(name="wt_ps", bufs=1, space="PSUM"))
        out_ps = ctx.enter_context(tc.tile_pool(name="out_ps", bufs=2, space="PSUM"))
        gp = ctx.enter_context(tc.tile_pool(name="gp", bufs=3))
        accp = ctx.enter_context(tc.tile_pool(name="accp", bufs=3))
        rhp = ctx.enter_context(tc.tile_pool(name="rhp", bufs=4))
        scr = ctx.enter_context(tc.tile_pool(name="scr", bufs=4))
        stg = ctx.enter_context(tc.tile_pool(name="stg", bufs=2))

        ident = sing.tile([128, 128], F32)
        make_identity(nc, ident[:])

        # ---- load weights ----
        w_om = sing.tile([C, K2, 48], F32R)
        nc.sync.dma_start(out=w_om[:], in_=w_om_in[:])
        b_om = sing.tile([48, 1], F32)
        nc.sync.dma_start(out=b_om[:], in_=b_om_in[:])
        w_cv = sing.tile([128, 5, CO], BF16)
        nc.sync.dma_start(out=w_cv[:], in_=w_cv_in[:])

        # ---- padded image (fp32) for convs ----
        xpad = sing.tile([C, HP * WP], F32R)
        nc.vector.memset(xpad[:].bitcast(F32), 0.0)
        xpad_rows = xpad[:].rearrange("c (h w) -> c h w", h=HP)
        nc.sync.dma_start(
            out=xpad_rows[:, 1:H + 1, 1:W + 1],
            in_=x_in[:].rearrange("c (h w) -> c h w", h=H),
        )

        # ---- offset/mask convs + transpose to w_t [wo, ho, ch] ----
        w_t = sing.tile([128, Ho, 48], F32)
        for t in range(16):
            ps = conv_ps.tile([48, 512], F32, tag="cps")
            for ki in range(K2):
                ky, kx = ki // 3, ki % 3
                rhs = bass.AP(
                    tensor=xpad.tensor,
                    offset=xpad[:].offset + (4 * t * SH + ky) * WP + kx,
                    ap=[list(xpad[:].ap[0]), [SH * WP, 4], [SW, 128]],
                )
                nc.tensor.matmul(
                    out=ps[0:41, :],
                    lhsT=w_om[:, ki, 0:41],
                    rhs=rhs,
                    start=(ki == 0),
                    stop=(ki == K2 - 1),
                )
            cstage = stg.tile([48, 4, 128], F32, tag="cstage")
            nc.scalar.activation(cstage[0:18, :, :], ps[0:18, :].rearrange("p (a b) -> p a b", a=4),
                                 AF.Identity, bias=b_om[0:18, :], scale=1.0)
            nc.scalar.activation(cstage[32:41, :, :], ps[32:41, :].rearrange("p (a b) -> p a b", a=4),
                                 AF.Sigmoid, bias=b_om[32:41, :], scale=1.0)
            for a in range(4):
                tp = wt_ps.tile([128, 48], F32, tag="wtp")
                nc.tensor.transpose(out=tp[:, 0:41], in_=cstage[0:41, a, :],
                                    identity=ident[0:41, 0:41])
                nc.vector.tensor_copy(out=w_t[:, 4 * t + a, 0:41], in_=tp[:, 0:41])

        # ---- sampling weights + gather indices ----
        # iotas
        i_ho_i = sing.tile([128, Ho], I32)
        nc.gpsimd.iota(i_ho_i[:], pattern=[[SH, Ho]], base=0, channel_multiplier=0)
        i_ho = sing.tile([128, Ho], F32)
        nc.vector.tensor_copy(out=i_ho[:], in_=i_ho_i[:])
        i_wo_i = sing.tile([128, 1], I32)
        nc.gpsimd.iota(i_wo_i[:], pattern=[[0, 1]], base=0, channel_multiplier=1)
        i_wo = sing.tile([128, 1], F32)
        nc.vector.tensor_copy(out=i_wo[:], in_=i_wo_i[:])

        py = sing.tile([128, Ho, K2], F32)
        px = sing.tile([128, Ho, K2], F32)
        # off_y channels 2k, off_x channels 2k+1 in w_t's ch dim
        for g in range(3):
            # ky group: k in {3g, 3g+1, 3g+2} -> off_y ch {6g, 6g+2, 6g+4}
            offy = bass.AP(tensor=w_t.tensor, offset=w_t[:].offset + 6 * g,
                          ap=[list(w_t[:].ap[0]), [48, Ho], [2, 3]])
            dsty = bass.AP(tensor=py.tensor, offset=py[:].offset + 3 * g,
                          ap=[list(py[:].ap[0]), [K2, Ho], [1, 3]])
            # py = off_y + (2*ho - PH + ky)
            nc.vector.tensor_tensor(out=dsty, in0=offy,
                                    in1=i_ho[:].to_broadcast([128, Ho, 3]), op=OP.add)
            nc.vector.tensor_scalar_add(dsty, dsty, float(g - PH))
            # kx group: k in {g, g+3, g+6} -> off_x ch {2g+1, 2g+7, 2g+13}
            offx = bass.AP(tensor=w_t.tensor, offset=w_t[:].offset + 2 * g + 1,
                          ap=[list(w_t[:].ap[0]), [48, Ho], [6, 3]])
            dstx = bass.AP(tensor=px.tensor, offset=px[:].offset + g,
                          ap=[list(px[:].ap[0]), [K2, Ho], [3, 3]])
            nc.vector.tensor_scalar_add(dstx, offx, i_wo[:, 0:1])
            nc.vector.tensor_scalar_add(dstx, dstx, float(g - PW))

        NF = Ho * K2  # 576
        pyf = py[:].rearrange("p a b -> p (a b)")
        pxf = px[:].rearrange("p a b -> p (a b)")

        def floor_frac(pos, fl, fr):
            fl_i = scr.tile([128, NF], I32, tag="scri")
            tmp = scr.tile([128, NF], F32, tag="scr")
            nc.vector.tensor_scalar_add(tmp[:], pos, 15.5)  # round-nearest(x+15.5)=floor(x)+16
            nc.vector.tensor_copy(out=fl_i[:], in_=tmp[:])   # f32 -> i32 trunc
            nc.vector.tensor_copy(out=fl[:], in_=fl_i[:])
            nc.vector.tensor_scalar_add(fl[:], fl[:], -16.0)
            nc.vector.tensor_tensor(out=fr[:], in0=pos, in1=fl[:], op=OP.subtract)

        y0f = sing.tile([128, NF], F32)
        fy = sing.tile([128, NF], F32)
        x0f = sing.tile([128, NF], F32)
        fx = sing.tile([128, NF], F32)
        floor_frac(pyf, y0f, fy)
        floor_frac(pxf, x0f, fx)

        def valid_mult(dst, v, lo, hi):
            a = scr.tile([128, NF], F32, tag="scr")
            nc.vector.tensor_scalar(a[:], v, lo, None, op0=OP.is_ge)
            nc.vector.tensor_tensor(out=dst, in0=dst, in1=a[:], op=OP.mult)
            nc.vector.tensor_scalar(a[:], v, hi, None, op0=OP.is_le)
            nc.vector.tensor_tensor(out=dst, in0=dst, in1=a[:], op=OP.mult)

        m_sig = sing.tile([128, NF], F32)
        msrc = bass.AP(tensor=w_t.tensor, offset=w_t[:].offset + 32,
                       ap=[list(w_t[:].ap[0]), [48, Ho], [1, K2]])
        nc.vector.tensor_copy(out=m_sig[:], in_=msrc)

        wy0 = sing.tile([128, NF], F32)
        nc.vector.tensor_scalar(wy0[:], fy[:], 1.0, -1.0, op0=OP.subtract, op1=OP.mult)
        nc.vector.tensor_tensor(out=wy0[:], in0=wy0[:], in1=m_sig[:], op=OP.mult)
        valid_mult(wy0[:], y0f[:], 0.0, float(H - 1))
        wy1 = sing.tile([128, NF], F32)
        nc.vector.tensor_tensor(out=wy1[:], in0=fy[:], in1=m_sig[:], op=OP.mult)
        valid_mult(wy1[:], y0f[:], -1.0, float(H - 2))
        wx0 = sing.tile([128, NF], F32)
        nc.vector.tensor_scalar(wx0[:], fx[:], 1.0, -1.0, op0=OP.subtract, op1=OP.mult)
        valid_mult(wx0[:], x0f[:], 0.0, float(W - 1))
        wx1 = sing.tile([128, NF], F32)
        nc.vector.tensor_copy(out=wx1[:], in_=fx[:])
        valid_mult(wx1[:], x0f[:], -1.0, float(W - 2))

        # interleaved corner-weight table wcat[wo, (ho,k), corner]
        wcat = sing.tile([128, NF * 4], F32)
        w00 = sing.tile([128, NF], F32)
        nc.vector.tensor_tensor(out=w00[:], in0=wy0[:], in1=wx0[:], op=OP.mult)
        w01 = sing.tile([128, NF], F32)
        nc.vector.tensor_tensor(out=w01[:], in0=wy0[:], in1=wx1[:], op=OP.mult)
        w10 = sing.tile([128, NF], F32)
        nc.vector.tensor_tensor(out=w10[:], in0=wy1[:], in1=wx0[:], op=OP.mult)
        w11 = sing.tile([128, NF], F32)
        nc.vector.tensor_tensor(out=w11[:], in0=wy1[:], in1=wx1[:], op=OP.mult)
        for ci, wt in enumerate((w00, w01, w10, w11)):
            dst = bass.AP(tensor=wcat.tensor, offset=wcat[:].offset + ci,
                          ap=[list(wcat[:].ap[0]), [4, NF]])
            nc.vector.tensor_copy(out=dst, in_=wt[:])

        idx = sing.tile([128, NF], I32)
        idf = scr.tile([128, NF], F32, tag="scr")
        nc.vector.tensor_scalar(idf[:], y0f[:], float(W), float(XQ_LEAD), op0=OP.mult, op1=OP.add)
        nc.vector.tensor_tensor(out=idf[:], in0=idf[:], in1=x0f[:], op=OP.add)
        nc.vector.tensor_scalar(idf[:], idf[:], 0.0, float(XQ_ROWS - 1), op0=OP.max, op1=OP.min)
        nc.vector.tensor_copy(out=idx[:], in_=idf[:])

        if dbg:
            nc.sync.dma_start(out=d_wt[:], in_=w_t[:])
            for i, wt in enumerate((w00, w01, w10, w11)):
                nc.sync.dma_start(out=d_w[i], in_=wt[:])
            nc.sync.dma_start(out=d_idx[:], in_=idx[:])

        ident_bf = sing.tile([128, 128], BF16)
        nc.vector.tensor_copy(out=ident_bf[:], in_=ident[:])

        # ---- main loop: per ho row, one 1152-descriptor gather + 3 wide
        # DVE/gpsimd elementwise ops + 5 (transpose, matmul) PE pairs ----
        NPOOL = 0  # taps whose corner-scale mult runs on gpsimd
        for ho in range(n_ho):
            s = ho * K2
            g = gp.tile([128, K2 * 256], BF16, tag="g")
            for ki in range(K2):
                nc.gpsimd.indirect_dma_start(
                    out=g[:, 256 * ki:256 * (ki + 1)], out_offset=None, in_=xq[:],
                    in_offset=bass.IndirectOffsetOnAxis(
                        ap=idx[:, s + ki:s + ki + 1], axis=0),
                )
            # tm[wo, (k,corner), c] = g * wcat (corner weights, bcast over c)
            tm = accp.tile([128, K2 * 256], BF16, tag="tm")
            nsp = NPOOL * 4 * 64
            ndv = K2 * 256 - nsp
            g3 = bass.AP(tensor=g.tensor, offset=g[:].offset,
                         ap=[list(g[:].ap[0]), [64, 4 * (K2 - NPOOL)], [1, 64]])
            w3 = bass.AP(tensor=wcat.tensor, offset=wcat[:].offset + s * 4,
                         ap=[list(wcat[:].ap[0]), [1, 4 * (K2 - NPOOL)], [0, 64]])
            t3 = bass.AP(tensor=tm.tensor, offset=tm[:].offset,
                         ap=[list(tm[:].ap[0]), [64, 4 * (K2 - NPOOL)], [1, 64]])
            nc.vector.tensor_tensor(out=t3, in0=g3, in1=w3, op=OP.mult)
            if NPOOL:
                g3p = bass.AP(tensor=g.tensor, offset=g[:].offset + ndv,
                              ap=[list(g[:].ap[0]), [64, 4 * NPOOL], [1, 64]])
                w3p = bass.AP(tensor=wcat.tensor,
                              offset=wcat[:].offset + s * 4 + 4 * (K2 - NPOOL),
                              ap=[list(wcat[:].ap[0]), [1, 4 * NPOOL], [0, 64]])
                t3p = bass.AP(tensor=tm.tensor, offset=tm[:].offset + ndv,
                              ap=[list(tm[:].ap[0]), [64, 4 * NPOOL], [1, 64]])
                nc.gpsimd.tensor_tensor(out=t3p, in0=g3p, in1=w3p, op=OP.mult)
            # u[wo, k, cc*c] = y0-pair + y1-pair
            u = rhp.tile([128, K2 * 128], BF16, tag="u")
            ua = bass.AP(tensor=tm.tensor, offset=tm[:].offset,
                         ap=[list(tm[:].ap[0]), [256, K2], [1, 128]])
            ub = bass.AP(tensor=tm.tensor, offset=tm[:].offset + 128,
                         ap=[list(tm[:].ap[0]), [256, K2], [1, 128]])
            uo = bass.AP(tensor=u.tensor, offset=u[:].offset,
                        ap=[list(u[:].ap[0]), [128, K2], [1, 128]])
            nc.vector.tensor_tensor(out=uo, in0=ua, in1=ub, op=OP.add)
            # acc[wo, k*64+c] = x0 + x1
            acc = accp.tile([128, 640], F32, tag="acc")
            va = bass.AP(tensor=u.tensor, offset=u[:].offset,
                         ap=[list(u[:].ap[0]), [128, K2], [1, 64]])
            vb = bass.AP(tensor=u.tensor, offset=u[:].offset + 64,
                         ap=[list(u[:].ap[0]), [128, K2], [1, 64]])
            vo = bass.AP(tensor=acc.tensor, offset=acc[:].offset,
                         ap=[list(acc[:].ap[0]), [64, K2], [1, 64]])
            nc.vector.tensor_tensor(out=vo, in0=va, in1=vb, op=OP.add)

            ops = out_ps.tile([CO, 128], F32, tag="ops")
            for half in range(5):
                tp = tr_ps.tile([128, 128], F32, tag="str")
                nc.tensor.transpose(out=tp[:], in_=acc[:, 128 * half:128 * (half + 1)],
                                    identity=ident[:])
                rh = rhp.tile([128, 128], BF16, tag="rh")
                nc.scalar.copy(out=rh[:], in_=tp[:])
                pdim = 128 if half < 4 else 64
                nc.tensor.matmul(
                    out=ops[:],
                    lhsT=w_cv[0:pdim, half, :],
                    rhs=rh[0:pdim, :],
                    start=(half == 0),
                    stop=(half == 4),
                )
            ob = rhp.tile([CO, 128], F32, tag="ob")
            nc.scalar.copy(out=ob[:], in_=ops[:])
            nc.sync.dma_start(
                out=out_ext[:].rearrange("o (ho wo) -> o ho wo", ho=Ho)[:, ho, :],
                in_=ob[:],
            )

    _split_sync_waits(nc)
    return nc


_CACHED = {}


def _get_runner(n_ho):
    if n_ho in _CACHED:
        return _CACHED[n_ho]
    _patch_tile_drain()
    nc = _build_nc(n_ho)
    from concourse.bass_utils import run_bass_kernel_spmd
    def run(in_maps, trace=False):
        return run_bass_kernel_spmd(nc, in_maps, list(range(N_CORES)), trace=trace)
    _CACHED[n_ho] = run
    return run


def _prep_weights(w_offset, b_offset, w_mask, b_mask, w_conv):
    import ml_dtypes
    w_om = np.zeros((C, K2, 48), np.float32)
    for ki in range(K2):
        ky, kx = ki // 3, ki % 3
        w_om[:, ki, 0:18] = w_offset[:, :, ky, kx].T
        w_om[:, ki, 32:41] = w_mask[:, :, ky, kx].T
    b_om = np.zeros((48, 1), np.float32)
    b_om[0:18, 0] = b_offset
    b_om[32:41, 0] = b_mask
    w_cv = np.zeros((128, 5, CO), np.float32)
    for half in range(5):
        k0 = 2 * half
        w_cv[0:64, half, :] = w_conv[:, :, k0 // 3, k0 % 3].T
        if k0 + 1 < K2:
            k1 = k0 + 1
            w_cv[64:128, half, :] = w_conv[:, :, k1 // 3, k1 % 3].T
    return w_om, b_om, w_cv.astype(ml_dtypes.bfloat16)


def _build_xq(xs):
    import ml_dtypes
    xt = np.zeros((XQ_LEAD + HWs + 384, C), np.float32)
    xt[XQ_LEAD:XQ_LEAD + HWs] = xs.reshape(C, HWs).T
    r0 = XQ_LEAD
    xqf = np.stack([xt[r0 - 129:r0 - 129 + XQ_ROWS],
                    xt[r0 - 128:r0 - 128 + XQ_ROWS],
                    xt[r0 - 1:r0 - 1 + XQ_ROWS],
                    xt[r0:r0 + XQ_ROWS]], axis=1)
    return np.ascontiguousarray(xqf.reshape(XQ_ROWS, 256).astype(ml_dtypes.bfloat16))


def kernel(x, w_offset, b_offset, w_mask, b_mask, w_conv, _trace=False):
    x = np.ascontiguousarray(np.asarray(x, np.float32))
    w_om, b_om, w_cv = _prep_weights(
        np.asarray(w_offset, np.float32), np.asarray(b_offset, np.float32),
        np.asarray(w_mask, np.float32), np.asarray(b_mask, np.float32),
        np.asarray(w_conv, np.float32))
    n_ho = int(os.environ.get("DEFC_N_HO", Ho))
    run = _get_runner(n_ho)
    in_maps = [
        {"x": x[c].reshape(C, H * W), "w_om": w_om, "b_om": b_om, "w_cv": w_cv,
         "xq": _build_xq(x[c])}
        for c in range(N_CORES)
    ]
    res = run(in_maps, trace=_trace)
    out = np.stack([res.results[c]["out"].reshape(CO, Ho, Wo) for c in range(N_CORES)])
    kernel._last_exec_ns = getattr(res, "exec_time_ns", None)
    return out

